# revision 32
# baseline (speedup 1.0000x reference)
"""Trainium2 Bass kernel for Chebyshev (L-inf) "convolution".

Math (see reference):
  out[b,co,h,w] = max_n |weights[co,n] - x_pad[b, c(co,n), h+di(co,n), w+dj(co,n)]| + bias[co]
  where conn_idx[co,n] = c*9 + di*3 + dj and x_pad is replicate-padded by 1.

Strategy (8 NeuronCores, batch-sharded: 4 images per core):
  1. Per image: load x contiguously into SBUF (8 KB descriptors), build a
     replicate-padded bf16 plane set [C=64, 66*66] via an on-engine strided
     cast-copy, store contiguously to DRAM scratch xpad_b.
  2. Per (image, tap): one indirect DMA; output partition co reads a
     contiguous 4222-element span of xpad_b starting at element offset
     idx[co] = c*4356 + di*66 + dj.  The shifted 64x64 window sits at
     row-stride 66 inside the span.
  3. Per tap: T_n = |G_n - w_n| -> bf16 (ScalarE Abs-activation with
     per-partition bias=-w).
  4. VectorE max tree over the 4 taps; store the bf16 max-plane; the host
     upcasts to fp32 and adds bias there (halves the out-store traffic;
     bf16 rounding of the max-term stays ~10x under the 2e-2 rel-err gate).
  5. Tail: the last image's tap 3 is gathered as two half-planes and its
     |.| runs on VectorE so the post-last-gather chain is short.

Built with Bacc so multi-wait instructions are legalized into event
semaphores (TRN2 allows 1 sync-wait per instruction).
"""

import numpy as np

B, CIN, H, W = 32, 64, 64, 64
COUT, NCONN = 128, 4
KH, KW = 3, 3
NCORES = 8
BL = B // NCORES            # 4 images per core
PH, PW = H + 2, W + 2       # 66 x 66 replicate-padded planes
PLANE = PH * PW             # 4356
S = H * W                   # 4096
SPAN = (H - 1) * PW + W     # 4222: span holding one shifted 64x64 window
GPAD = SPAN + 2             # 4224 (even) SBUF tile width
HH = H // 2                 # 32 rows per half-plane (tail pipelining)
SPANH = (HH - 1) * PW + W   # 2110: span of a 32x64 half window
GPADH = SPANH + 2           # 2112
NSLOT = BL * NCONN + 1      # 17 offset slots (16 taps + tap3 half-B)

_CACHE = {}


def _build_program():
    import concourse.bass as bass
    import concourse.bacc as bacc
    import concourse.mybir as mybir
    from concourse.tile import TileContext, add_dep_helper

    f32 = mybir.dt.float32
    bf16 = mybir.dt.bfloat16
    i32 = mybir.dt.int32
    Alu = mybir.AluOpType
    Act = mybir.ActivationFunctionType

    nc = bacc.Bacc("TRN2", target_bir_lowering=False, debug=False)

    x_ext = nc.dram_tensor("x", (BL, CIN, H, W), f32, kind="ExternalInput").ap()
    # cols 0..3 = -w (ScalarE Abs bias), cols 4..7 = +w (VectorE w-g path)
    wneg_ext = nc.dram_tensor(
        "wneg", (COUT, 2 * NCONN), f32, kind="ExternalInput"
    ).ap()
    # per (b, n): one 8-int32 slot per partition at cols [(b*NCONN+n)*8, +8);
    # the indirect-DMA ucode reads col 0 (the rest pad the 32 B block).
    gidx_ext = nc.dram_tensor(
        "gidx", (COUT, NSLOT * 8), i32, kind="ExternalInput"
    ).ap()
    out_ext = [
        nc.dram_tensor(f"out{b}", (COUT, H, W), bf16, kind="ExternalOutput").ap()
        for b in range(BL)
    ]
    xpads = [
        nc.dram_tensor(f"xpad{b}", (CIN * PLANE, 1), bf16) for b in range(BL)
    ]

    with TileContext(nc, pool_alloc_mode="queue") as tc:
        with (
            tc.tile_pool(name="const", bufs=1) as cpool,
            tc.tile_pool(name="xp", bufs=2) as xpool,
            tc.tile_pool(name="g", bufs=5) as gpool,
            tc.tile_pool(name="t", bufs=4) as tpool,
            tc.tile_pool(name="m", bufs=2) as mpool,
        ):
            wneg_sb = cpool.tile([COUT, 2 * NCONN], f32)
            nc.sync.dma_start(out=wneg_sb[:], in_=wneg_ext)
            gidx_sb = cpool.tile([COUT, NSLOT * 8], i32)
            nc.sync.dma_start(out=gidx_sb[:], in_=gidx_ext)

            for b in range(BL):
                # --- padded bf16 planes for image b ---
                XSB = xpool.tile([CIN, S], f32, tag="xsb")
                xv = x_ext[b].rearrange("c h w -> c (h w)")
                # quarter-loads -> 4 KB descriptors (big ones read slow)
                for qq in range(4):
                    qsl = slice(qq * (S // 4), (qq + 1) * (S // 4))
                    nc.sync.dma_start(out=XSB[:, qsl], in_=xv[:, qsl])
                XP = xpool.tile([CIN, PLANE], bf16, tag="xp")
                XPv = XP[:].rearrange("c (h w) -> c h w", h=PH, w=PW)
                nc.vector.tensor_copy(
                    out=XPv[:, 1 : H + 1, 1 : W + 1],
                    in_=XSB[:].rearrange("c (h w) -> c h w", h=H, w=W),
                )
                nc.vector.tensor_copy(
                    out=XPv[:, 1 : H + 1, 0:1], in_=XPv[:, 1 : H + 1, 1:2]
                )
                nc.vector.tensor_copy(
                    out=XPv[:, 1 : H + 1, PW - 1 : PW],
                    in_=XPv[:, 1 : H + 1, PW - 2 : PW - 1],
                )
                nc.vector.tensor_copy(out=XPv[:, 0:1, :], in_=XPv[:, 1:2, :])
                nc.vector.tensor_copy(
                    out=XPv[:, PH - 1 : PH, :], in_=XPv[:, PH - 2 : PH - 1, :]
                )
                # contiguous store of the padded planes (8.7 KiB/partition)
                store = nc.sync.dma_start(
                    out=xpads[b].ap().rearrange(
                        "(c p) one -> c (p one)", c=CIN, p=PLANE
                    ),
                    in_=XP[:],
                )

                # --- full-span taps: gather + |G - w| (ScalarE) ---
                last = b == BL - 1
                nfull = NCONN if not last else 0
                ts = []
                for n in range(nfull):
                    k = b * NCONN + n
                    gt = gpool.tile([COUT, GPAD], bf16, tag="g")
                    gather = nc.gpsimd.indirect_dma_start(
                        out=gt[:, 0:SPAN],
                        out_offset=None,
                        in_=xpads[b].ap(),
                        in_offset=bass.IndirectOffsetOnAxis(
                            ap=gidx_sb[:, k * 8 : k * 8 + 1], axis=0
                        ),
                    )
                    add_dep_helper(
                        gather.ins, store.ins, reason="gather reads xpad[b]"
                    )
                    gv = gt[:].rearrange("p (h w) -> p h w", h=H, w=PW)[:, :, 0:W]
                    tt = tpool.tile([COUT, S], bf16, tag="t")
                    tv = tt[:].rearrange("p (h w) -> p h w", h=H, w=W)
                    nc.scalar.activation(
                        out=tv,
                        in_=gv,
                        func=Act.Abs,
                        bias=wneg_sb[:, n : n + 1],
                        scale=1.0,
                    )
                    ts.append(tt)

                outv = out_ext[b].rearrange("c h w -> c (h w)")
                if not last:
                    # --- max tree (VectorE), bf16 half-stores ---
                    m0 = mpool.tile([COUT, S], bf16, tag="m")
                    nc.vector.tensor_tensor(
                        out=m0[:], in0=ts[0][:], in1=ts[1][:], op=Alu.max
                    )
                    m1 = mpool.tile([COUT, S], bf16, tag="m")
                    nc.vector.tensor_tensor(
                        out=m1[:], in0=ts[2][:], in1=ts[3][:], op=Alu.max
                    )
                    for hh in range(2):
                        sl = slice(hh * (S // 2), (hh + 1) * (S // 2))
                        m2 = mpool.tile([COUT, S // 2], bf16, tag="m2", bufs=3)
                        nc.vector.tensor_tensor(
                            out=m2[:], in0=m0[:, sl], in1=m1[:, sl], op=Alu.max
                        )
                        nc.sync.dma_start(out=outv[:, sl], in_=m2[:])
                    continue
                # --- last image: all 5 gathers issued up front (halves
                # first); half |G-w| split V(g-w) / Pool(w-g); tap2 ABS in
                # halves on ScalarE; progressive max chain so only one max
                # per half remains after the final ABS ---
                def igather(slot, width, span):
                    gt = gpool.tile(
                        [COUT, width], bf16,
                        tag="g" if width == GPAD else "gh",
                        bufs=5 if width == GPAD else 2,
                        name="gt",
                    )
                    g = nc.gpsimd.indirect_dma_start(
                        out=gt[:, 0:span],
                        out_offset=None,
                        in_=xpads[b].ap(),
                        in_offset=bass.IndirectOffsetOnAxis(
                            ap=gidx_sb[:, slot * 8 : slot * 8 + 1], axis=0
                        ),
                    )
                    add_dep_helper(
                        g.ins, store.ins, reason="gather reads xpad[b]"
                    )
                    return gt

                gh_t = [
                    igather(b * NCONN + 3, GPADH, SPANH),
                    igather(BL * NCONN, GPADH, SPANH),
                ]
                gf = [
                    igather(b * NCONN + n, GPAD, SPAN) for n in range(3)
                ]

                # half |G - w3| chains (early; gathers already in flight)
                t3h = []
                for hh in range(2):
                    gv = gh_t[hh][:].rearrange(
                        "p (h w) -> p h w", h=HH, w=PW
                    )[:, :, 0:W]
                    d3 = tpool.tile([COUT, S // 2], bf16, tag="d", bufs=2)
                    d3v = d3[:].rearrange("p (h w) -> p h w", h=HH, w=W)
                    nc.vector.tensor_scalar(
                        out=d3v,
                        in0=gv,
                        scalar1=wneg_sb[:, 3:4],
                        scalar2=None,
                        op0=Alu.add,
                    )
                    e3 = tpool.tile([COUT, S // 2], bf16, tag="e", bufs=2)
                    e3v = e3[:].rearrange("p (h w) -> p h w", h=HH, w=W)
                    nc.gpsimd.tensor_scalar(
                        out=e3v,
                        in0=gv,
                        scalar1=-1.0,
                        scalar2=wneg_sb[:, 7:8],
                        op0=Alu.mult,
                        op1=Alu.add,
                    )
                    t3 = tpool.tile([COUT, S // 2], bf16, tag="th", bufs=2)
                    nc.vector.tensor_tensor(
                        out=t3[:], in0=d3[:], in1=e3[:], op=Alu.max
                    )
                    t3h.append(t3)

                # taps 0, 1: full ABS (ScalarE) + progressive max per half
                pcur = t3h
                for n in range(2):
                    gv = gf[n][:].rearrange(
                        "p (h w) -> p h w", h=H, w=PW
                    )[:, :, 0:W]
                    tt = tpool.tile([COUT, S], bf16, tag="t", name="tt")
                    tv = tt[:].rearrange("p (h w) -> p h w", h=H, w=W)
                    nc.scalar.activation(
                        out=tv,
                        in_=gv,
                        func=Act.Abs,
                        bias=wneg_sb[:, n : n + 1],
                        scale=1.0,
                    )
                    pnew = []
                    for hh in range(2):
                        sl = slice(hh * (S // 2), (hh + 1) * (S // 2))
                        pp = tpool.tile(
                            [COUT, S // 2], bf16, tag="ph", bufs=4, name="pp"
                        )
                        nc.vector.tensor_tensor(
                            out=pp[:], in0=pcur[hh][:], in1=tt[:, sl],
                            op=Alu.max,
                        )
                        pnew.append(pp)
                    pcur = pnew

                # tap 2: full gather, ABS per half (ScalarE), final max+store
                gv2 = gf[2][:].rearrange("p (h w) -> p h w", h=H, w=PW)[
                    :, :, 0:W
                ]
                tt2 = tpool.tile([COUT, S], bf16, tag="t")
                tv2 = tt2[:].rearrange("p (h w) -> p h w", h=H, w=W)
                for hh in range(2):
                    rs = slice(hh * HH, (hh + 1) * HH)
                    nc.scalar.activation(
                        out=tv2[:, rs, :],
                        in_=gv2[:, rs, :],
                        func=Act.Abs,
                        bias=wneg_sb[:, 2:3],
                        scale=1.0,
                    )
                    sl0 = hh * (S // 2)
                    nq = 2 if hh == 1 else 1
                    for q in range(nq):
                        qw = S // 2 // nq
                        qsl = slice(q * qw, (q + 1) * qw)
                        osl = slice(sl0 + q * qw, sl0 + (q + 1) * qw)
                        m2 = mpool.tile([COUT, qw], bf16, tag="m2", bufs=3)
                        nc.vector.tensor_tensor(
                            out=m2[:],
                            in0=pcur[hh][:, qsl],
                            in1=tt2[:, osl],
                            op=Alu.max,
                        )
                        nc.sync.dma_start(out=outv[:, osl], in_=m2[:])
    nc.compile()
    return nc


def _host_inputs(x, weights, bias, conn_idx):
    """Per-core input maps (host-side prep: shard x, derive -w / gather
    row-indices from the tiny weight/index tensors)."""
    ci = np.asarray(conn_idx).astype(np.int64)          # [COUT, NCONN]
    c = ci // (KH * KW)
    rem = ci % (KH * KW)
    di = rem // KW
    dj = rem % KW
    # element offset into xpad_b [64, 66, 66]: c*4356 + di*66 + dj
    offs = (c * PLANE + di * PW + dj).astype(np.int32)          # [COUT, NCONN]
    gidx = np.zeros((COUT, NSLOT * 8), dtype=np.int32)
    for bb in range(BL):
        for n in range(NCONN):
            k = bb * NCONN + n
            gidx[:, k * 8] = offs[:, n]
    # slot BL*NCONN: half-B (rows 32..63) of the last image's tap 3
    gidx[:, BL * NCONN * 8] = offs[:, 3] + HH * PW
    w = np.asarray(weights).astype(np.float32)
    wneg = np.concatenate([-w, w], axis=1)              # [COUT, 2*NCONN]
    x = np.ascontiguousarray(np.asarray(x), dtype=np.float32)
    in_maps = []
    for kcore in range(NCORES):
        in_maps.append(
            {
                "x": x[kcore * BL : (kcore + 1) * BL],
                "wneg": wneg,
                "gidx": gidx,
            }
        )
    return in_maps


def kernel(x, weights, bias, conn_idx):
    from concourse.bass_utils import run_bass_kernel_spmd

    if "nc" not in _CACHE:
        _CACHE["nc"] = _build_program()
    nc = _CACHE["nc"]
    in_maps = _host_inputs(x, weights, bias, conn_idx)
    res = run_bass_kernel_spmd(nc, in_maps, list(range(NCORES)))
    outs = [
        np.stack(
            [
                np.asarray(res.results[k][f"out{b}"]).astype(np.float32)
                for b in range(BL)
            ]
        )
        for k in range(NCORES)
    ]
    full = np.concatenate(outs, axis=0)
    # bias added on host in fp32 (device ships the bf16 max-term only)
    full += np.asarray(bias).reshape(1, COUT, 1, 1).astype(np.float32)
    return full


if __name__ == "__main__":
    nc = _build_program()
    print("program built OK")


# revision 33
# speedup vs baseline: 1.1259x; 1.1259x over previous
"""Trainium2 Bass kernel for Chebyshev (L-inf) "convolution".

Math (see reference):
  out[b,co,h,w] = max_n |weights[co,n] - x_pad[b, c(co,n), h+di(co,n), w+dj(co,n)]| + bias[co]
  where conn_idx[co,n] = c*9 + di*3 + dj and x_pad is replicate-padded by 1.

Strategy (8 NeuronCores, batch-sharded: 4 images per core):
  1. Per image: load x contiguously into SBUF (8 KB descriptors), build a
     replicate-padded bf16 plane set [C=64, 66*66] via an on-engine strided
     cast-copy, store contiguously to DRAM scratch xpad_b.
  2. Per (image, tap): one indirect DMA; output partition co reads a
     contiguous 4222-element span of xpad_b starting at element offset
     idx[co] = c*4356 + di*66 + dj.  The shifted 64x64 window sits at
     row-stride 66 inside the span.
  3. Per tap: T_n = |G_n - w_n| -> bf16 (ScalarE Abs-activation with
     per-partition bias=-w).
  4. VectorE max tree over the 4 taps; store the bf16 max-plane; the host
     upcasts to fp32 and adds bias there (halves the out-store traffic;
     bf16 rounding of the max-term stays ~10x under the 2e-2 rel-err gate).
  5. Tail: the last image's tap 3 is gathered as two half-planes and its
     |.| runs on VectorE so the post-last-gather chain is short.

Built with Bacc so multi-wait instructions are legalized into event
semaphores (TRN2 allows 1 sync-wait per instruction).
"""

import numpy as np

B, CIN, H, W = 32, 64, 64, 64
COUT, NCONN = 128, 4
KH, KW = 3, 3
NCORES = 8
BL = B // NCORES            # 4 images per core
PH, PW = H + 2, W + 2       # 66 x 66 replicate-padded planes
PLANE = PH * PW             # 4356
S = H * W                   # 4096
SPAN = (H - 1) * PW + W     # 4222: span holding one shifted 64x64 window
GPAD = SPAN + 2             # 4224 (even) SBUF tile width
HH = H // 2                 # 32 rows per half-plane (tail pipelining)
SPANH = (HH - 1) * PW + W   # 2110: span of a 32x64 half window
GPADH = SPANH + 2           # 2112
NSLOT = BL * NCONN + 1      # 17 offset slots (16 taps + tap3 half-B)

_CACHE = {}


def _build_program():
    import concourse.bass as bass
    import concourse.bacc as bacc
    import concourse.mybir as mybir
    from concourse.tile import TileContext, add_dep_helper

    f32 = mybir.dt.float32
    bf16 = mybir.dt.bfloat16
    i32 = mybir.dt.int32
    Alu = mybir.AluOpType
    Act = mybir.ActivationFunctionType

    nc = bacc.Bacc("TRN2", target_bir_lowering=False, debug=False)

    x_ext = nc.dram_tensor("x", (BL, CIN, H, W), f32, kind="ExternalInput").ap()
    # cols 0..3 = -w (ScalarE Abs bias), cols 4..7 = +w (VectorE w-g path)
    wneg_ext = nc.dram_tensor(
        "wneg", (COUT, 2 * NCONN), f32, kind="ExternalInput"
    ).ap()
    # per (b, n): one 8-int32 slot per partition at cols [(b*NCONN+n)*8, +8);
    # the indirect-DMA ucode reads col 0 (the rest pad the 32 B block).
    gidx_ext = nc.dram_tensor(
        "gidx", (COUT, NSLOT * 8), i32, kind="ExternalInput"
    ).ap()
    out_ext = [
        nc.dram_tensor(f"out{b}", (COUT, H, W), bf16, kind="ExternalOutput").ap()
        for b in range(BL)
    ]
    xpads = [
        nc.dram_tensor(f"xpad{b}", (CIN * PLANE, 1), bf16) for b in range(BL)
    ]

    with TileContext(nc, pool_alloc_mode="queue") as tc:
        with (
            tc.tile_pool(name="const", bufs=1) as cpool,
            tc.tile_pool(name="xp", bufs=2) as xpool,
            tc.tile_pool(name="g", bufs=5) as gpool,
            tc.tile_pool(name="t", bufs=4) as tpool,
            tc.tile_pool(name="m", bufs=2) as mpool,
        ):
            wneg_sb = cpool.tile([COUT, 2 * NCONN], f32)
            nc.sync.dma_start(out=wneg_sb[:], in_=wneg_ext)
            gidx_sb = cpool.tile([COUT, NSLOT * 8], i32)
            nc.sync.dma_start(out=gidx_sb[:], in_=gidx_ext)

            def build_pair(b0):
                """Build xpad for images b0, b0+1 as one [128, .] tile
                (partition = image x channel): 128-partition load/cast and
                half the V instruction count; returns the two store DMAs."""
                XSB = xpool.tile([2 * CIN, S], f32, tag="xsb")
                xv = x_ext[b0 : b0 + 2].rearrange("b c h w -> (b c) (h w)")
                # quarter-loads -> 4 KB descriptors (big ones read slow)
                for qq in range(4):
                    qsl = slice(qq * (S // 4), (qq + 1) * (S // 4))
                    nc.sync.dma_start(out=XSB[:, qsl], in_=xv[:, qsl])
                XP = xpool.tile([2 * CIN, PLANE], bf16, tag="xp")
                XPv = XP[:].rearrange("p (h w) -> p h w", h=PH, w=PW)
                nc.vector.tensor_copy(
                    out=XPv[:, 1 : H + 1, 1 : W + 1],
                    in_=XSB[:].rearrange("p (h w) -> p h w", h=H, w=W),
                )
                nc.vector.tensor_copy(
                    out=XPv[:, 1 : H + 1, 0:1], in_=XPv[:, 1 : H + 1, 1:2]
                )
                nc.vector.tensor_copy(
                    out=XPv[:, 1 : H + 1, PW - 1 : PW],
                    in_=XPv[:, 1 : H + 1, PW - 2 : PW - 1],
                )
                nc.vector.tensor_copy(out=XPv[:, 0:1, :], in_=XPv[:, 1:2, :])
                nc.vector.tensor_copy(
                    out=XPv[:, PH - 1 : PH, :], in_=XPv[:, PH - 2 : PH - 1, :]
                )
                outs = []
                for k in range(2):
                    outs.append(
                        nc.sync.dma_start(
                            out=xpads[b0 + k].ap().rearrange(
                                "(c p) one -> c (p one)", c=CIN, p=PLANE
                            ),
                            in_=XP[k * CIN : (k + 1) * CIN, :],
                        )
                    )
                return outs

            stores = {}
            for b in range(BL):
                if b % 2 == 0:
                    stores[b], stores[b + 1] = build_pair(b)
                store = stores[b]

                # --- full-span taps: gather + |G - w| (ScalarE) ---
                last = b == BL - 1
                nfull = NCONN if not last else 0
                ts = []
                for n in range(nfull):
                    k = b * NCONN + n
                    gt = gpool.tile([COUT, GPAD], bf16, tag="g")
                    gather = nc.gpsimd.indirect_dma_start(
                        out=gt[:, 0:SPAN],
                        out_offset=None,
                        in_=xpads[b].ap(),
                        in_offset=bass.IndirectOffsetOnAxis(
                            ap=gidx_sb[:, k * 8 : k * 8 + 1], axis=0
                        ),
                    )
                    add_dep_helper(
                        gather.ins, store.ins, reason="gather reads xpad[b]"
                    )
                    gv = gt[:].rearrange("p (h w) -> p h w", h=H, w=PW)[:, :, 0:W]
                    tt = tpool.tile([COUT, S], bf16, tag="t")
                    tv = tt[:].rearrange("p (h w) -> p h w", h=H, w=W)
                    nc.scalar.activation(
                        out=tv,
                        in_=gv,
                        func=Act.Abs,
                        bias=wneg_sb[:, n : n + 1],
                        scale=1.0,
                    )
                    ts.append(tt)

                outv = out_ext[b].rearrange("c h w -> c (h w)")
                if not last:
                    # --- max tree (VectorE), bf16 half-stores ---
                    m0 = mpool.tile([COUT, S], bf16, tag="m")
                    nc.vector.tensor_tensor(
                        out=m0[:], in0=ts[0][:], in1=ts[1][:], op=Alu.max
                    )
                    m1 = mpool.tile([COUT, S], bf16, tag="m")
                    nc.vector.tensor_tensor(
                        out=m1[:], in0=ts[2][:], in1=ts[3][:], op=Alu.max
                    )
                    for hh in range(2):
                        sl = slice(hh * (S // 2), (hh + 1) * (S // 2))
                        m2 = mpool.tile([COUT, S // 2], bf16, tag="m2", bufs=3)
                        nc.vector.tensor_tensor(
                            out=m2[:], in0=m0[:, sl], in1=m1[:, sl], op=Alu.max
                        )
                        nc.sync.dma_start(out=outv[:, sl], in_=m2[:])
                    continue
                # --- last image: all 5 gathers issued up front (halves
                # first); half |G-w| split V(g-w) / Pool(w-g); tap2 ABS in
                # halves on ScalarE; progressive max chain so only one max
                # per half remains after the final ABS ---
                def igather(slot, width, span):
                    gt = gpool.tile(
                        [COUT, width], bf16,
                        tag="g" if width == GPAD else "gh",
                        bufs=5 if width == GPAD else 2,
                        name="gt",
                    )
                    g = nc.gpsimd.indirect_dma_start(
                        out=gt[:, 0:span],
                        out_offset=None,
                        in_=xpads[b].ap(),
                        in_offset=bass.IndirectOffsetOnAxis(
                            ap=gidx_sb[:, slot * 8 : slot * 8 + 1], axis=0
                        ),
                    )
                    add_dep_helper(
                        g.ins, store.ins, reason="gather reads xpad[b]"
                    )
                    return gt

                gh_t = [
                    igather(b * NCONN + 3, GPADH, SPANH),
                    igather(BL * NCONN, GPADH, SPANH),
                ]
                gf = [
                    igather(b * NCONN + n, GPAD, SPAN) for n in range(3)
                ]

                # half |G - w3| chains (early; gathers already in flight)
                t3h = []
                for hh in range(2):
                    gv = gh_t[hh][:].rearrange(
                        "p (h w) -> p h w", h=HH, w=PW
                    )[:, :, 0:W]
                    d3 = tpool.tile([COUT, S // 2], bf16, tag="d", bufs=2)
                    d3v = d3[:].rearrange("p (h w) -> p h w", h=HH, w=W)
                    nc.vector.tensor_scalar(
                        out=d3v,
                        in0=gv,
                        scalar1=wneg_sb[:, 3:4],
                        scalar2=None,
                        op0=Alu.add,
                    )
                    e3 = tpool.tile([COUT, S // 2], bf16, tag="e", bufs=2)
                    e3v = e3[:].rearrange("p (h w) -> p h w", h=HH, w=W)
                    nc.gpsimd.tensor_scalar(
                        out=e3v,
                        in0=gv,
                        scalar1=-1.0,
                        scalar2=wneg_sb[:, 7:8],
                        op0=Alu.mult,
                        op1=Alu.add,
                    )
                    t3 = tpool.tile([COUT, S // 2], bf16, tag="th", bufs=2)
                    nc.vector.tensor_tensor(
                        out=t3[:], in0=d3[:], in1=e3[:], op=Alu.max
                    )
                    t3h.append(t3)

                # taps 0, 1: full ABS (ScalarE) + progressive max per half
                pcur = t3h
                for n in range(2):
                    gv = gf[n][:].rearrange(
                        "p (h w) -> p h w", h=H, w=PW
                    )[:, :, 0:W]
                    tt = tpool.tile([COUT, S], bf16, tag="t", name="tt")
                    tv = tt[:].rearrange("p (h w) -> p h w", h=H, w=W)
                    nc.scalar.activation(
                        out=tv,
                        in_=gv,
                        func=Act.Abs,
                        bias=wneg_sb[:, n : n + 1],
                        scale=1.0,
                    )
                    pnew = []
                    for hh in range(2):
                        sl = slice(hh * (S // 2), (hh + 1) * (S // 2))
                        pp = tpool.tile(
                            [COUT, S // 2], bf16, tag="ph", bufs=4, name="pp"
                        )
                        nc.vector.tensor_tensor(
                            out=pp[:], in0=pcur[hh][:], in1=tt[:, sl],
                            op=Alu.max,
                        )
                        pnew.append(pp)
                    pcur = pnew

                # tap 2: full gather, ABS per half (ScalarE), final max+store
                gv2 = gf[2][:].rearrange("p (h w) -> p h w", h=H, w=PW)[
                    :, :, 0:W
                ]
                tt2 = tpool.tile([COUT, S], bf16, tag="t")
                tv2 = tt2[:].rearrange("p (h w) -> p h w", h=H, w=W)
                for hh in range(2):
                    rs = slice(hh * HH, (hh + 1) * HH)
                    nc.scalar.activation(
                        out=tv2[:, rs, :],
                        in_=gv2[:, rs, :],
                        func=Act.Abs,
                        bias=wneg_sb[:, 2:3],
                        scale=1.0,
                    )
                    sl0 = hh * (S // 2)
                    nq = 2 if hh == 1 else 1
                    for q in range(nq):
                        qw = S // 2 // nq
                        qsl = slice(q * qw, (q + 1) * qw)
                        osl = slice(sl0 + q * qw, sl0 + (q + 1) * qw)
                        m2 = mpool.tile([COUT, qw], bf16, tag="m2", bufs=3)
                        nc.vector.tensor_tensor(
                            out=m2[:],
                            in0=pcur[hh][:, qsl],
                            in1=tt2[:, osl],
                            op=Alu.max,
                        )
                        nc.sync.dma_start(out=outv[:, osl], in_=m2[:])
    nc.compile()
    return nc


def _host_inputs(x, weights, bias, conn_idx):
    """Per-core input maps (host-side prep: shard x, derive -w / gather
    row-indices from the tiny weight/index tensors)."""
    ci = np.asarray(conn_idx).astype(np.int64)          # [COUT, NCONN]
    c = ci // (KH * KW)
    rem = ci % (KH * KW)
    di = rem // KW
    dj = rem % KW
    # element offset into xpad_b [64, 66, 66]: c*4356 + di*66 + dj
    offs = (c * PLANE + di * PW + dj).astype(np.int32)          # [COUT, NCONN]
    gidx = np.zeros((COUT, NSLOT * 8), dtype=np.int32)
    for bb in range(BL):
        for n in range(NCONN):
            k = bb * NCONN + n
            gidx[:, k * 8] = offs[:, n]
    # slot BL*NCONN: half-B (rows 32..63) of the last image's tap 3
    gidx[:, BL * NCONN * 8] = offs[:, 3] + HH * PW
    w = np.asarray(weights).astype(np.float32)
    wneg = np.concatenate([-w, w], axis=1)              # [COUT, 2*NCONN]
    x = np.ascontiguousarray(np.asarray(x), dtype=np.float32)
    in_maps = []
    for kcore in range(NCORES):
        in_maps.append(
            {
                "x": x[kcore * BL : (kcore + 1) * BL],
                "wneg": wneg,
                "gidx": gidx,
            }
        )
    return in_maps


def kernel(x, weights, bias, conn_idx):
    from concourse.bass_utils import run_bass_kernel_spmd

    if "nc" not in _CACHE:
        _CACHE["nc"] = _build_program()
    nc = _CACHE["nc"]
    in_maps = _host_inputs(x, weights, bias, conn_idx)
    res = run_bass_kernel_spmd(nc, in_maps, list(range(NCORES)))
    outs = [
        np.stack(
            [
                np.asarray(res.results[k][f"out{b}"]).astype(np.float32)
                for b in range(BL)
            ]
        )
        for k in range(NCORES)
    ]
    full = np.concatenate(outs, axis=0)
    # bias added on host in fp32 (device ships the bf16 max-term only)
    full += np.asarray(bias).reshape(1, COUT, 1, 1).astype(np.float32)
    return full


if __name__ == "__main__":
    nc = _build_program()
    print("program built OK")


# revision 34
# speedup vs baseline: 1.2122x; 1.0767x over previous
"""Trainium2 Bass kernel for Chebyshev (L-inf) "convolution".

Math (see reference):
  out[b,co,h,w] = max_n |weights[co,n] - x_pad[b, c(co,n), h+di(co,n), w+dj(co,n)]| + bias[co]
  where conn_idx[co,n] = c*9 + di*3 + dj and x_pad is replicate-padded by 1.

Strategy (8 NeuronCores, batch-sharded: 4 images per core):
  1. Per image: load x contiguously into SBUF (8 KB descriptors), build a
     replicate-padded bf16 plane set [C=64, 66*66] via an on-engine strided
     cast-copy, store contiguously to DRAM scratch xpad_b.
  2. Per (image, tap): one indirect DMA; output partition co reads a
     contiguous 4222-element span of xpad_b starting at element offset
     idx[co] = c*4356 + di*66 + dj.  The shifted 64x64 window sits at
     row-stride 66 inside the span.
  3. Per tap: T_n = |G_n - w_n| -> bf16 (ScalarE Abs-activation with
     per-partition bias=-w).
  4. VectorE max tree over the 4 taps; store the bf16 max-plane; the host
     upcasts to fp32 and adds bias there (halves the out-store traffic;
     bf16 rounding of the max-term stays ~10x under the 2e-2 rel-err gate).
  5. Tail: the last image's tap 3 is gathered as two half-planes and its
     |.| runs on VectorE so the post-last-gather chain is short.

Built with Bacc so multi-wait instructions are legalized into event
semaphores (TRN2 allows 1 sync-wait per instruction).
"""

import numpy as np

B, CIN, H, W = 32, 64, 64, 64
COUT, NCONN = 128, 4
KH, KW = 3, 3
NCORES = 8
BL = B // NCORES            # 4 images per core
PH, PW = H + 2, W + 2       # 66 x 66 replicate-padded planes
PLANE = PH * PW             # 4356
S = H * W                   # 4096
SPAN = (H - 1) * PW + W     # 4222: span holding one shifted 64x64 window
GPAD = SPAN + 2             # 4224 (even) SBUF tile width
HH = H // 2                 # 32 rows per half-plane (tail pipelining)
SPANH = (HH - 1) * PW + W   # 2110: span of a 32x64 half window
GPADH = SPANH + 2           # 2112
NSLOT = BL * NCONN + 1      # 17 offset slots (16 taps + tap3 half-B)

_CACHE = {}


def _build_program():
    import concourse.bass as bass
    import concourse.bacc as bacc
    import concourse.mybir as mybir
    from concourse.tile import TileContext, add_dep_helper

    f32 = mybir.dt.float32
    bf16 = mybir.dt.bfloat16
    i32 = mybir.dt.int32
    Alu = mybir.AluOpType
    Act = mybir.ActivationFunctionType

    nc = bacc.Bacc("TRN2", target_bir_lowering=False, debug=False)

    x_ext = nc.dram_tensor("x", (BL, CIN, H, W), f32, kind="ExternalInput").ap()
    # cols 0..3 = -w (ScalarE Abs bias), cols 4..7 = +w (VectorE w-g path)
    wneg_ext = nc.dram_tensor(
        "wneg", (COUT, 2 * NCONN), f32, kind="ExternalInput"
    ).ap()
    # per (b, n): one 8-int32 slot per partition at cols [(b*NCONN+n)*8, +8);
    # the indirect-DMA ucode reads col 0 (the rest pad the 32 B block).
    gidx_ext = nc.dram_tensor(
        "gidx", (COUT, NSLOT * 8), i32, kind="ExternalInput"
    ).ap()
    out_ext = [
        nc.dram_tensor(f"out{b}", (COUT, H, W), bf16, kind="ExternalOutput").ap()
        for b in range(BL)
    ]
    # one scratch tensor per image PAIR: [(k c) plane] so the pad-build
    # stores with a single [128, .] DMA (64-partition reads run ~2x slow)
    xpads = [
        nc.dram_tensor(f"xpad{p}", (2 * CIN * PLANE, 1), bf16)
        for p in range(BL // 2)
    ]

    with TileContext(nc, pool_alloc_mode="queue") as tc:
        with (
            tc.tile_pool(name="const", bufs=1) as cpool,
            tc.tile_pool(name="xp", bufs=2) as xpool,
            tc.tile_pool(name="g", bufs=5) as gpool,
            tc.tile_pool(name="t", bufs=4) as tpool,
            tc.tile_pool(name="m", bufs=2) as mpool,
        ):
            wneg_sb = cpool.tile([COUT, 2 * NCONN], f32)
            nc.sync.dma_start(out=wneg_sb[:], in_=wneg_ext)
            gidx_sb = cpool.tile([COUT, NSLOT * 8], i32)
            nc.sync.dma_start(out=gidx_sb[:], in_=gidx_ext)

            def build_pair(b0):
                """Build xpad for images b0, b0+1 as one [128, .] tile
                (partition = image x channel): 128-partition load/cast and
                half the V instruction count; returns the two store DMAs."""
                XSB = xpool.tile([2 * CIN, S], f32, tag="xsb")
                xv = x_ext[b0 : b0 + 2].rearrange("b c h w -> (b c) (h w)")
                # quarter-loads -> 4 KB descriptors (big ones read slow)
                for qq in range(4):
                    qsl = slice(qq * (S // 4), (qq + 1) * (S // 4))
                    nc.sync.dma_start(out=XSB[:, qsl], in_=xv[:, qsl])
                XP = xpool.tile([2 * CIN, PLANE], bf16, tag="xp")
                XPv = XP[:].rearrange("p (h w) -> p h w", h=PH, w=PW)
                nc.vector.tensor_copy(
                    out=XPv[:, 1 : H + 1, 1 : W + 1],
                    in_=XSB[:].rearrange("p (h w) -> p h w", h=H, w=W),
                )
                nc.vector.tensor_copy(
                    out=XPv[:, 1 : H + 1, 0:1], in_=XPv[:, 1 : H + 1, 1:2]
                )
                nc.vector.tensor_copy(
                    out=XPv[:, 1 : H + 1, PW - 1 : PW],
                    in_=XPv[:, 1 : H + 1, PW - 2 : PW - 1],
                )
                nc.vector.tensor_copy(out=XPv[:, 0:1, :], in_=XPv[:, 1:2, :])
                nc.vector.tensor_copy(
                    out=XPv[:, PH - 1 : PH, :], in_=XPv[:, PH - 2 : PH - 1, :]
                )
                st = nc.sync.dma_start(
                    out=xpads[b0 // 2].ap().rearrange(
                        "(c p) one -> c (p one)", c=2 * CIN, p=PLANE
                    ),
                    in_=XP[:],
                )
                return st, st

            stores = {}
            for b in range(BL):
                if b % 2 == 0:
                    stores[b], stores[b + 1] = build_pair(b)
                store = stores[b]

                # --- full-span taps: gather + |G - w| (ScalarE) ---
                last = b == BL - 1
                nfull = NCONN if not last else 0
                ts = []
                for n in range(nfull):
                    k = b * NCONN + n
                    gt = gpool.tile([COUT, GPAD], bf16, tag="g")
                    gather = nc.gpsimd.indirect_dma_start(
                        out=gt[:, 0:SPAN],
                        out_offset=None,
                        in_=xpads[b // 2].ap(),
                        in_offset=bass.IndirectOffsetOnAxis(
                            ap=gidx_sb[:, k * 8 : k * 8 + 1], axis=0
                        ),
                    )
                    add_dep_helper(
                        gather.ins, store.ins, reason="gather reads xpad[b]"
                    )
                    gv = gt[:].rearrange("p (h w) -> p h w", h=H, w=PW)[:, :, 0:W]
                    tt = tpool.tile([COUT, S], bf16, tag="t")
                    tv = tt[:].rearrange("p (h w) -> p h w", h=H, w=W)
                    nc.scalar.activation(
                        out=tv,
                        in_=gv,
                        func=Act.Abs,
                        bias=wneg_sb[:, n : n + 1],
                        scale=1.0,
                    )
                    ts.append(tt)

                outv = out_ext[b].rearrange("c h w -> c (h w)")
                if not last:
                    # --- max tree (VectorE), bf16 half-stores ---
                    m0 = mpool.tile([COUT, S], bf16, tag="m")
                    nc.vector.tensor_tensor(
                        out=m0[:], in0=ts[0][:], in1=ts[1][:], op=Alu.max
                    )
                    m1 = mpool.tile([COUT, S], bf16, tag="m")
                    nc.vector.tensor_tensor(
                        out=m1[:], in0=ts[2][:], in1=ts[3][:], op=Alu.max
                    )
                    for hh in range(2):
                        sl = slice(hh * (S // 2), (hh + 1) * (S // 2))
                        m2 = mpool.tile([COUT, S // 2], bf16, tag="m2", bufs=3)
                        nc.vector.tensor_tensor(
                            out=m2[:], in0=m0[:, sl], in1=m1[:, sl], op=Alu.max
                        )
                        nc.sync.dma_start(out=outv[:, sl], in_=m2[:])
                    continue
                # --- last image: all 5 gathers issued up front (halves
                # first); half |G-w| split V(g-w) / Pool(w-g); tap2 ABS in
                # halves on ScalarE; progressive max chain so only one max
                # per half remains after the final ABS ---
                def igather(slot, width, span):
                    gt = gpool.tile(
                        [COUT, width], bf16,
                        tag="g" if width == GPAD else "gh",
                        bufs=5 if width == GPAD else 2,
                        name="gt",
                    )
                    g = nc.gpsimd.indirect_dma_start(
                        out=gt[:, 0:span],
                        out_offset=None,
                        in_=xpads[b // 2].ap(),
                        in_offset=bass.IndirectOffsetOnAxis(
                            ap=gidx_sb[:, slot * 8 : slot * 8 + 1], axis=0
                        ),
                    )
                    add_dep_helper(
                        g.ins, store.ins, reason="gather reads xpad[b]"
                    )
                    return gt

                gh_t = [
                    igather(b * NCONN + 3, GPADH, SPANH),
                    igather(BL * NCONN, GPADH, SPANH),
                ]
                gf = [
                    igather(b * NCONN + n, GPAD, SPAN) for n in range(3)
                ]

                # half |G - w3| chains (early; gathers already in flight)
                t3h = []
                for hh in range(2):
                    gv = gh_t[hh][:].rearrange(
                        "p (h w) -> p h w", h=HH, w=PW
                    )[:, :, 0:W]
                    d3 = tpool.tile([COUT, S // 2], bf16, tag="d", bufs=2)
                    d3v = d3[:].rearrange("p (h w) -> p h w", h=HH, w=W)
                    nc.vector.tensor_scalar(
                        out=d3v,
                        in0=gv,
                        scalar1=wneg_sb[:, 3:4],
                        scalar2=None,
                        op0=Alu.add,
                    )
                    e3 = tpool.tile([COUT, S // 2], bf16, tag="e", bufs=2)
                    e3v = e3[:].rearrange("p (h w) -> p h w", h=HH, w=W)
                    nc.gpsimd.tensor_scalar(
                        out=e3v,
                        in0=gv,
                        scalar1=-1.0,
                        scalar2=wneg_sb[:, 7:8],
                        op0=Alu.mult,
                        op1=Alu.add,
                    )
                    t3 = tpool.tile([COUT, S // 2], bf16, tag="th", bufs=2)
                    nc.vector.tensor_tensor(
                        out=t3[:], in0=d3[:], in1=e3[:], op=Alu.max
                    )
                    t3h.append(t3)

                # taps 0, 1: full ABS (ScalarE) + progressive max per half
                pcur = t3h
                for n in range(2):
                    gv = gf[n][:].rearrange(
                        "p (h w) -> p h w", h=H, w=PW
                    )[:, :, 0:W]
                    tt = tpool.tile([COUT, S], bf16, tag="t", name="tt")
                    tv = tt[:].rearrange("p (h w) -> p h w", h=H, w=W)
                    nc.scalar.activation(
                        out=tv,
                        in_=gv,
                        func=Act.Abs,
                        bias=wneg_sb[:, n : n + 1],
                        scale=1.0,
                    )
                    pnew = []
                    for hh in range(2):
                        sl = slice(hh * (S // 2), (hh + 1) * (S // 2))
                        pp = tpool.tile(
                            [COUT, S // 2], bf16, tag="ph", bufs=4, name="pp"
                        )
                        nc.vector.tensor_tensor(
                            out=pp[:], in0=pcur[hh][:], in1=tt[:, sl],
                            op=Alu.max,
                        )
                        pnew.append(pp)
                    pcur = pnew

                # tap 2: full gather, ABS per half (ScalarE), final max+store
                gv2 = gf[2][:].rearrange("p (h w) -> p h w", h=H, w=PW)[
                    :, :, 0:W
                ]
                tt2 = tpool.tile([COUT, S], bf16, tag="t")
                tv2 = tt2[:].rearrange("p (h w) -> p h w", h=H, w=W)
                for hh in range(2):
                    rs = slice(hh * HH, (hh + 1) * HH)
                    nc.scalar.activation(
                        out=tv2[:, rs, :],
                        in_=gv2[:, rs, :],
                        func=Act.Abs,
                        bias=wneg_sb[:, 2:3],
                        scale=1.0,
                    )
                    sl0 = hh * (S // 2)
                    nq = 2 if hh == 1 else 1
                    for q in range(nq):
                        qw = S // 2 // nq
                        qsl = slice(q * qw, (q + 1) * qw)
                        osl = slice(sl0 + q * qw, sl0 + (q + 1) * qw)
                        m2 = mpool.tile([COUT, qw], bf16, tag="m2", bufs=3)
                        nc.vector.tensor_tensor(
                            out=m2[:],
                            in0=pcur[hh][:, qsl],
                            in1=tt2[:, osl],
                            op=Alu.max,
                        )
                        nc.sync.dma_start(out=outv[:, osl], in_=m2[:])
    nc.compile()
    return nc


def _host_inputs(x, weights, bias, conn_idx):
    """Per-core input maps (host-side prep: shard x, derive -w / gather
    row-indices from the tiny weight/index tensors)."""
    ci = np.asarray(conn_idx).astype(np.int64)          # [COUT, NCONN]
    c = ci // (KH * KW)
    rem = ci % (KH * KW)
    di = rem // KW
    dj = rem % KW
    # element offset into xpad_b [64, 66, 66]: c*4356 + di*66 + dj
    offs = (c * PLANE + di * PW + dj).astype(np.int32)          # [COUT, NCONN]
    gidx = np.zeros((COUT, NSLOT * 8), dtype=np.int32)
    for bb in range(BL):
        for n in range(NCONN):
            k = bb * NCONN + n
            gidx[:, k * 8] = offs[:, n] + (bb % 2) * CIN * PLANE
    # slot BL*NCONN: half-B (rows 32..63) of the last image's tap 3
    # (last image has odd parity within its pair)
    gidx[:, BL * NCONN * 8] = offs[:, 3] + HH * PW + CIN * PLANE
    w = np.asarray(weights).astype(np.float32)
    wneg = np.concatenate([-w, w], axis=1)              # [COUT, 2*NCONN]
    x = np.ascontiguousarray(np.asarray(x), dtype=np.float32)
    in_maps = []
    for kcore in range(NCORES):
        in_maps.append(
            {
                "x": x[kcore * BL : (kcore + 1) * BL],
                "wneg": wneg,
                "gidx": gidx,
            }
        )
    return in_maps


def kernel(x, weights, bias, conn_idx):
    from concourse.bass_utils import run_bass_kernel_spmd

    if "nc" not in _CACHE:
        _CACHE["nc"] = _build_program()
    nc = _CACHE["nc"]
    in_maps = _host_inputs(x, weights, bias, conn_idx)
    res = run_bass_kernel_spmd(nc, in_maps, list(range(NCORES)))
    outs = [
        np.stack(
            [
                np.asarray(res.results[k][f"out{b}"]).astype(np.float32)
                for b in range(BL)
            ]
        )
        for k in range(NCORES)
    ]
    full = np.concatenate(outs, axis=0)
    # bias added on host in fp32 (device ships the bf16 max-term only)
    full += np.asarray(bias).reshape(1, COUT, 1, 1).astype(np.float32)
    return full


if __name__ == "__main__":
    nc = _build_program()
    print("program built OK")


# revision 35
# speedup vs baseline: 1.2463x; 1.0281x over previous
"""Trainium2 Bass kernel for Chebyshev (L-inf) "convolution".

Math (see reference):
  out[b,co,h,w] = max_n |weights[co,n] - x_pad[b, c(co,n), h+di(co,n), w+dj(co,n)]| + bias[co]
  where conn_idx[co,n] = c*9 + di*3 + dj and x_pad is replicate-padded by 1.

Strategy (8 NeuronCores, batch-sharded: 4 images per core):
  1. Per image: load x contiguously into SBUF (8 KB descriptors), build a
     replicate-padded bf16 plane set [C=64, 66*66] via an on-engine strided
     cast-copy, store contiguously to DRAM scratch xpad_b.
  2. Per (image, tap): one indirect DMA; output partition co reads a
     contiguous 4222-element span of xpad_b starting at element offset
     idx[co] = c*4356 + di*66 + dj.  The shifted 64x64 window sits at
     row-stride 66 inside the span.
  3. Per tap: T_n = |G_n - w_n| -> bf16 (ScalarE Abs-activation with
     per-partition bias=-w).
  4. VectorE max tree over the 4 taps; store the bf16 max-plane; the host
     upcasts to fp32 and adds bias there (halves the out-store traffic;
     bf16 rounding of the max-term stays ~10x under the 2e-2 rel-err gate).
  5. Tail: the last image's tap 3 is gathered as two half-planes and its
     |.| runs on VectorE so the post-last-gather chain is short.

Built with Bacc so multi-wait instructions are legalized into event
semaphores (TRN2 allows 1 sync-wait per instruction).
"""

import numpy as np

B, CIN, H, W = 32, 64, 64, 64
COUT, NCONN = 128, 4
KH, KW = 3, 3
NCORES = 8
BL = B // NCORES            # 4 images per core
PH, PW = H + 2, W + 2       # 66 x 66 replicate-padded planes
PLANE = PH * PW             # 4356
S = H * W                   # 4096
SPAN = (H - 1) * PW + W     # 4222: span holding one shifted 64x64 window
GPAD = SPAN + 2             # 4224 (even) SBUF tile width
HH = H // 2                 # 32 rows per half-plane (tail pipelining)
SPANH = (HH - 1) * PW + W   # 2110: span of a 32x64 half window
GPADH = SPANH + 2           # 2112
NSLOT = BL * NCONN + 1      # 17 offset slots (16 taps + tap3 half-B)

_CACHE = {}


def _build_program():
    import concourse.bass as bass
    import concourse.bacc as bacc
    import concourse.mybir as mybir
    from concourse.tile import TileContext, add_dep_helper

    f32 = mybir.dt.float32
    bf16 = mybir.dt.bfloat16
    i32 = mybir.dt.int32
    Alu = mybir.AluOpType
    Act = mybir.ActivationFunctionType

    nc = bacc.Bacc("TRN2", target_bir_lowering=False, debug=False)

    x_ext = nc.dram_tensor("x", (BL, CIN, H, W), f32, kind="ExternalInput").ap()
    # cols 0..3 = -w (ScalarE Abs bias), cols 4..7 = +w (VectorE w-g path)
    wneg_ext = nc.dram_tensor(
        "wneg", (COUT, 2 * NCONN), f32, kind="ExternalInput"
    ).ap()
    # per (b, n): one 8-int32 slot per partition at cols [(b*NCONN+n)*8, +8);
    # the indirect-DMA ucode reads col 0 (the rest pad the 32 B block).
    gidx_ext = nc.dram_tensor(
        "gidx", (COUT, NSLOT * 8), i32, kind="ExternalInput"
    ).ap()
    out_ext = [
        nc.dram_tensor(f"out{b}", (COUT, H, W), bf16, kind="ExternalOutput").ap()
        for b in range(BL)
    ]
    # one scratch tensor per image PAIR: [(k c) plane] so the pad-build
    # stores with a single [128, .] DMA (64-partition reads run ~2x slow)
    xpads = [
        nc.dram_tensor(f"xpad{p}", (2 * CIN * PLANE, 1), bf16)
        for p in range(BL // 2)
    ]

    with TileContext(nc, pool_alloc_mode="queue") as tc:
        with (
            tc.tile_pool(name="const", bufs=1) as cpool,
            tc.tile_pool(name="xp", bufs=2) as xpool,
            tc.tile_pool(name="g", bufs=5) as gpool,
            tc.tile_pool(name="t", bufs=4) as tpool,
            tc.tile_pool(name="m", bufs=2) as mpool,
        ):
            wneg_sb = cpool.tile([COUT, 2 * NCONN], f32)
            nc.sync.dma_start(out=wneg_sb[:], in_=wneg_ext)
            gidx_sb = cpool.tile([COUT, NSLOT * 8], i32)
            nc.sync.dma_start(out=gidx_sb[:], in_=gidx_ext)

            def build_pair(b0):
                """Build xpad for images b0, b0+1 as one [128, .] tile
                (partition = image x channel): 128-partition load/cast and
                half the V instruction count; returns the two store DMAs."""
                XSB = xpool.tile([2 * CIN, S], f32, tag="xsb")
                xv = x_ext[b0 : b0 + 2].rearrange("b c h w -> (b c) (h w)")
                # quarter-loads -> 4 KB descriptors (big ones read slow)
                for qq in range(4):
                    qsl = slice(qq * (S // 4), (qq + 1) * (S // 4))
                    nc.sync.dma_start(out=XSB[:, qsl], in_=xv[:, qsl])
                XP = xpool.tile([2 * CIN, PLANE], bf16, tag="xp")
                XPv = XP[:].rearrange("p (h w) -> p h w", h=PH, w=PW)
                nc.vector.tensor_copy(
                    out=XPv[:, 1 : H + 1, 1 : W + 1],
                    in_=XSB[:].rearrange("p (h w) -> p h w", h=H, w=W),
                )
                nc.vector.tensor_copy(
                    out=XPv[:, 1 : H + 1, 0:1], in_=XPv[:, 1 : H + 1, 1:2]
                )
                nc.vector.tensor_copy(
                    out=XPv[:, 1 : H + 1, PW - 1 : PW],
                    in_=XPv[:, 1 : H + 1, PW - 2 : PW - 1],
                )
                nc.vector.tensor_copy(out=XPv[:, 0:1, :], in_=XPv[:, 1:2, :])
                nc.vector.tensor_copy(
                    out=XPv[:, PH - 1 : PH, :], in_=XPv[:, PH - 2 : PH - 1, :]
                )
                st = nc.sync.dma_start(
                    out=xpads[b0 // 2].ap().rearrange(
                        "(c p) one -> c (p one)", c=2 * CIN, p=PLANE
                    ),
                    in_=XP[:],
                )
                return st, st

            # both pair-builds up front: their loads enter the DMA queues
            # before the sections' out-store triggers (SyncE is in-order),
            # filling the early-stream DMA slack
            stores = {}
            stores[0], stores[1] = build_pair(0)
            stores[2], stores[3] = build_pair(2)
            for b in range(BL):
                store = stores[b]

                # --- full-span taps: gather + |G - w| (ScalarE) ---
                last = b == BL - 1
                nfull = NCONN if not last else 0
                ts = []
                for n in range(nfull):
                    k = b * NCONN + n
                    gt = gpool.tile([COUT, GPAD], bf16, tag="g")
                    gather = nc.gpsimd.indirect_dma_start(
                        out=gt[:, 0:SPAN],
                        out_offset=None,
                        in_=xpads[b // 2].ap(),
                        in_offset=bass.IndirectOffsetOnAxis(
                            ap=gidx_sb[:, k * 8 : k * 8 + 1], axis=0
                        ),
                    )
                    add_dep_helper(
                        gather.ins, store.ins, reason="gather reads xpad[b]"
                    )
                    gv = gt[:].rearrange("p (h w) -> p h w", h=H, w=PW)[:, :, 0:W]
                    tt = tpool.tile([COUT, S], bf16, tag="t")
                    tv = tt[:].rearrange("p (h w) -> p h w", h=H, w=W)
                    nc.scalar.activation(
                        out=tv,
                        in_=gv,
                        func=Act.Abs,
                        bias=wneg_sb[:, n : n + 1],
                        scale=1.0,
                    )
                    ts.append(tt)

                outv = out_ext[b].rearrange("c h w -> c (h w)")
                if not last:
                    # --- max tree (VectorE), bf16 half-stores ---
                    m0 = mpool.tile([COUT, S], bf16, tag="m")
                    nc.vector.tensor_tensor(
                        out=m0[:], in0=ts[0][:], in1=ts[1][:], op=Alu.max
                    )
                    m1 = mpool.tile([COUT, S], bf16, tag="m")
                    nc.vector.tensor_tensor(
                        out=m1[:], in0=ts[2][:], in1=ts[3][:], op=Alu.max
                    )
                    for hh in range(2):
                        sl = slice(hh * (S // 2), (hh + 1) * (S // 2))
                        m2 = mpool.tile([COUT, S // 2], bf16, tag="m2", bufs=3)
                        nc.vector.tensor_tensor(
                            out=m2[:], in0=m0[:, sl], in1=m1[:, sl], op=Alu.max
                        )
                        nc.sync.dma_start(out=outv[:, sl], in_=m2[:])
                    continue
                # --- last image: all 5 gathers issued up front (halves
                # first); half |G-w| split V(g-w) / Pool(w-g); tap2 ABS in
                # halves on ScalarE; progressive max chain so only one max
                # per half remains after the final ABS ---
                def igather(slot, width, span):
                    gt = gpool.tile(
                        [COUT, width], bf16,
                        tag="g" if width == GPAD else "gh",
                        bufs=5 if width == GPAD else 2,
                        name="gt",
                    )
                    g = nc.gpsimd.indirect_dma_start(
                        out=gt[:, 0:span],
                        out_offset=None,
                        in_=xpads[b // 2].ap(),
                        in_offset=bass.IndirectOffsetOnAxis(
                            ap=gidx_sb[:, slot * 8 : slot * 8 + 1], axis=0
                        ),
                    )
                    add_dep_helper(
                        g.ins, store.ins, reason="gather reads xpad[b]"
                    )
                    return gt

                gh_t = [
                    igather(b * NCONN + 3, GPADH, SPANH),
                    igather(BL * NCONN, GPADH, SPANH),
                ]
                gf = [
                    igather(b * NCONN + n, GPAD, SPAN) for n in range(3)
                ]

                # half |G - w3| chains (early; gathers already in flight)
                t3h = []
                for hh in range(2):
                    gv = gh_t[hh][:].rearrange(
                        "p (h w) -> p h w", h=HH, w=PW
                    )[:, :, 0:W]
                    d3 = tpool.tile([COUT, S // 2], bf16, tag="d", bufs=2)
                    d3v = d3[:].rearrange("p (h w) -> p h w", h=HH, w=W)
                    nc.vector.tensor_scalar(
                        out=d3v,
                        in0=gv,
                        scalar1=wneg_sb[:, 3:4],
                        scalar2=None,
                        op0=Alu.add,
                    )
                    e3 = tpool.tile([COUT, S // 2], bf16, tag="e", bufs=2)
                    e3v = e3[:].rearrange("p (h w) -> p h w", h=HH, w=W)
                    nc.gpsimd.tensor_scalar(
                        out=e3v,
                        in0=gv,
                        scalar1=-1.0,
                        scalar2=wneg_sb[:, 7:8],
                        op0=Alu.mult,
                        op1=Alu.add,
                    )
                    t3 = tpool.tile([COUT, S // 2], bf16, tag="th", bufs=2)
                    nc.vector.tensor_tensor(
                        out=t3[:], in0=d3[:], in1=e3[:], op=Alu.max
                    )
                    t3h.append(t3)

                # taps 0, 1: full ABS (ScalarE) + progressive max per half
                pcur = t3h
                for n in range(2):
                    gv = gf[n][:].rearrange(
                        "p (h w) -> p h w", h=H, w=PW
                    )[:, :, 0:W]
                    tt = tpool.tile([COUT, S], bf16, tag="t", name="tt")
                    tv = tt[:].rearrange("p (h w) -> p h w", h=H, w=W)
                    nc.scalar.activation(
                        out=tv,
                        in_=gv,
                        func=Act.Abs,
                        bias=wneg_sb[:, n : n + 1],
                        scale=1.0,
                    )
                    pnew = []
                    for hh in range(2):
                        sl = slice(hh * (S // 2), (hh + 1) * (S // 2))
                        pp = tpool.tile(
                            [COUT, S // 2], bf16, tag="ph", bufs=4, name="pp"
                        )
                        nc.vector.tensor_tensor(
                            out=pp[:], in0=pcur[hh][:], in1=tt[:, sl],
                            op=Alu.max,
                        )
                        pnew.append(pp)
                    pcur = pnew

                # tap 2: full gather, ABS per half (ScalarE), final max+store
                gv2 = gf[2][:].rearrange("p (h w) -> p h w", h=H, w=PW)[
                    :, :, 0:W
                ]
                tt2 = tpool.tile([COUT, S], bf16, tag="t")
                tv2 = tt2[:].rearrange("p (h w) -> p h w", h=H, w=W)
                for hh in range(2):
                    rs = slice(hh * HH, (hh + 1) * HH)
                    nc.scalar.activation(
                        out=tv2[:, rs, :],
                        in_=gv2[:, rs, :],
                        func=Act.Abs,
                        bias=wneg_sb[:, 2:3],
                        scale=1.0,
                    )
                    sl0 = hh * (S // 2)
                    nq = 2 if hh == 1 else 1
                    for q in range(nq):
                        qw = S // 2 // nq
                        qsl = slice(q * qw, (q + 1) * qw)
                        osl = slice(sl0 + q * qw, sl0 + (q + 1) * qw)
                        m2 = mpool.tile([COUT, qw], bf16, tag="m2", bufs=3)
                        nc.vector.tensor_tensor(
                            out=m2[:],
                            in0=pcur[hh][:, qsl],
                            in1=tt2[:, osl],
                            op=Alu.max,
                        )
                        nc.sync.dma_start(out=outv[:, osl], in_=m2[:])
    nc.compile()
    return nc


def _host_inputs(x, weights, bias, conn_idx):
    """Per-core input maps (host-side prep: shard x, derive -w / gather
    row-indices from the tiny weight/index tensors)."""
    ci = np.asarray(conn_idx).astype(np.int64)          # [COUT, NCONN]
    c = ci // (KH * KW)
    rem = ci % (KH * KW)
    di = rem // KW
    dj = rem % KW
    # element offset into xpad_b [64, 66, 66]: c*4356 + di*66 + dj
    offs = (c * PLANE + di * PW + dj).astype(np.int32)          # [COUT, NCONN]
    gidx = np.zeros((COUT, NSLOT * 8), dtype=np.int32)
    for bb in range(BL):
        for n in range(NCONN):
            k = bb * NCONN + n
            gidx[:, k * 8] = offs[:, n] + (bb % 2) * CIN * PLANE
    # slot BL*NCONN: half-B (rows 32..63) of the last image's tap 3
    # (last image has odd parity within its pair)
    gidx[:, BL * NCONN * 8] = offs[:, 3] + HH * PW + CIN * PLANE
    w = np.asarray(weights).astype(np.float32)
    wneg = np.concatenate([-w, w], axis=1)              # [COUT, 2*NCONN]
    x = np.ascontiguousarray(np.asarray(x), dtype=np.float32)
    in_maps = []
    for kcore in range(NCORES):
        in_maps.append(
            {
                "x": x[kcore * BL : (kcore + 1) * BL],
                "wneg": wneg,
                "gidx": gidx,
            }
        )
    return in_maps


def kernel(x, weights, bias, conn_idx):
    from concourse.bass_utils import run_bass_kernel_spmd

    if "nc" not in _CACHE:
        _CACHE["nc"] = _build_program()
    nc = _CACHE["nc"]
    in_maps = _host_inputs(x, weights, bias, conn_idx)
    res = run_bass_kernel_spmd(nc, in_maps, list(range(NCORES)))
    outs = [
        np.stack(
            [
                np.asarray(res.results[k][f"out{b}"]).astype(np.float32)
                for b in range(BL)
            ]
        )
        for k in range(NCORES)
    ]
    full = np.concatenate(outs, axis=0)
    # bias added on host in fp32 (device ships the bf16 max-term only)
    full += np.asarray(bias).reshape(1, COUT, 1, 1).astype(np.float32)
    return full


if __name__ == "__main__":
    nc = _build_program()
    print("program built OK")
